# revision 1
# baseline (speedup 1.0000x reference)
"""Trainium2 Bass kernel for CausalSelfAttention (GQA + qk-rmsnorm + rope + head gating).

Sharding: 8 cores = 2 (batch) x 4 (kv-head groups). Each core computes the
full attention for one batch element and one kv-head group (4 q heads), plus
its slice of the output projection; partial projection outputs are summed on
the host.

Per-core on-device pipeline (all matmuls bf16 with fp32 PSUM accumulation):
  A) fused QKV+gate projection -> rmsnorm stats + rope (DVE/ACT) ->
     DMA-transpose q,k into head-dim-major layout
  B) flash-style causal attention per head in S^T layout:
     S^T = K @ Q^T, P = exp(S/sqrt(d)) (no max subtraction: |logits| <= 11.3),
     diagonal-block masking, Y = P @ [V | 1] (ones column gives the softmax
     denominator for free), per-token normalize * sigmoid gate,
     DMA-transpose y
  C) output projection partial: out = y @ Wproj_slice^T
"""

import numpy as np
import ml_dtypes
from contextlib import ExitStack

import concourse.bass as bass
import concourse.bacc as bacc
import concourse.mybir as mybir
import concourse.tile as tile
from concourse.bass_utils import run_bass_kernel_spmd

BF16 = mybir.dt.bfloat16
F32 = mybir.dt.float32
NPBF = ml_dtypes.bfloat16

B, T, D = 2, 2048, 2048
H, HKV, HD = 16, 4, 128
HALF = HD // 2
NHEAD = H // HKV          # q heads per core (group)
NT = T // 128             # 16 token tiles
NCHUNK = D // 128         # 16 contraction chunks
NQKV = NHEAD * HD + HD + HD + NHEAD   # 512 q + 128 k + 128 v + 4 gate = 772
ROPE_BASE = 10000.0
EPS = float(np.finfo(np.float32).eps)
SM_SCALE = 1.0 / float(np.sqrt(HD))

_CACHE = {}


def _build_program():
    nc = bacc.Bacc("TRN2", target_bir_lowering=False, debug=False,
                   enable_asserts=False, num_devices=8)

    xT_d = nc.dram_tensor("xT", [D, T], BF16, kind="ExternalInput").ap()
    wqkvg_d = nc.dram_tensor("wqkvg", [D, NQKV], BF16, kind="ExternalInput").ap()
    wproj_d = nc.dram_tensor("wproj", [NHEAD * HD, D], BF16, kind="ExternalInput").ap()
    cos_d = nc.dram_tensor("cosd", [T, HALF], F32, kind="ExternalInput").ap()
    sin_d = nc.dram_tensor("sind", [T, HALF], F32, kind="ExternalInput").ap()
    qgain_d = nc.dram_tensor("qgain", [1, NHEAD], F32, kind="ExternalInput").ap()
    gateb_d = nc.dram_tensor("gateb", [1, NHEAD], F32, kind="ExternalInput").ap()
    masks_d = nc.dram_tensor("masks", [128, 4, 512], BF16, kind="ExternalInput").ap()
    out_d = nc.dram_tensor("out", [T, D], F32, kind="ExternalOutput").ap()

    AF = mybir.ActivationFunctionType

    with tile.TileContext(nc) as tc, ExitStack() as ctx:
        consts = ctx.enter_context(tc.tile_pool(name="consts", bufs=1))

        # ---- resident tensors ----
        xT_sb = consts.tile([128, NCHUNK, T], BF16)
        for c in range(NCHUNK):
            nc.sync.dma_start(out=xT_sb[:, c, :], in_=xT_d[c * 128:(c + 1) * 128, :])
        wqkvg_sb = consts.tile([128, NCHUNK, NQKV], BF16)
        for c in range(NCHUNK):
            nc.sync.dma_start(out=wqkvg_sb[:, c, :],
                              in_=wqkvg_d[c * 128:(c + 1) * 128, :])
        wproj_sb = consts.tile([128, NHEAD, D], BF16)
        for h in range(NHEAD):
            nc.sync.dma_start(out=wproj_sb[:, h, :],
                              in_=wproj_d[h * 128:(h + 1) * 128, :])
        cos_sb = consts.tile([128, NT, HALF], F32)
        nc.sync.dma_start(out=cos_sb,
                          in_=cos_d.rearrange("(tt p) i -> p tt i", p=128))
        sin_sb = consts.tile([128, NT, HALF], F32)
        nc.sync.dma_start(out=sin_sb,
                          in_=sin_d.rearrange("(tt p) i -> p tt i", p=128))
        qgain_sb = consts.tile([128, NHEAD], F32)
        nc.sync.dma_start(out=qgain_sb, in_=bass.AP(
            tensor=qgain_d.tensor, offset=qgain_d.offset,
            ap=[[0, 128], [1, NHEAD]]))
        gateb_sb = consts.tile([128, NHEAD], F32)
        nc.sync.dma_start(out=gateb_sb, in_=bass.AP(
            tensor=gateb_d.tensor, offset=gateb_d.offset,
            ap=[[0, 128], [1, NHEAD]]))
        masks_sb = consts.tile([128, 4, 512], BF16)
        nc.sync.dma_start(out=masks_sb, in_=masks_d)

        qT_sb = consts.tile([128, NHEAD, T], BF16)   # head-dim-major q
        kT_sb = consts.tile([128, T], BF16)          # head-dim-major k
        v_sb = consts.tile([128, NT, HD + 1], BF16)  # [t | ones] per ki tile
        nc.vector.memset(v_sb[:, :, HD:HD + 1], 1.0)
        yT_sb = consts.tile([128, NHEAD, T], BF16)   # head-dim-major gated y
        gate_sb = consts.tile([128, NT, NHEAD], F32)
        eps_sb = consts.tile([128, 1], F32)
        nc.vector.memset(eps_sb, EPS)

        # =========== Phase A: QKV + gate, rms stats, rope, transpose ==========
        a_sb = ctx.enter_context(tc.tile_pool(name="phA", bufs=2))
        with tc.tile_pool(name="phA_ps", bufs=2, space="PSUM") as a_ps:
          for tg in range(NT // 4):
            glog_g = a_sb.tile([128, 4, NHEAD], F32, tag="glog_g")
            msq_g = a_sb.tile([128, 4, NHEAD + 1], F32, tag="msq_g")
            qst_g = a_sb.tile([128, 4, NHEAD, HD], BF16, tag="qst_g")
            kst_g = a_sb.tile([128, 4, HD], BF16, tag="kst_g")
            for ti in range(4):
                tt = tg * 4 + ti
                ts = slice(tt * 128, (tt + 1) * 128)
                qkv_a = a_ps.tile([128, 512], F32, tag="qkv_a")
                qkv_b = a_ps.tile([128, NQKV - 512], F32, tag="qkv_b")
                for c in range(NCHUNK):
                    lhs = xT_sb[:, c, ts]
                    nc.tensor.matmul(qkv_a, lhsT=lhs, rhs=wqkvg_sb[:, c, 0:512],
                                     start=(c == 0), stop=(c == NCHUNK - 1))
                    nc.tensor.matmul(qkv_b, lhsT=lhs, rhs=wqkvg_sb[:, c, 512:NQKV],
                                     start=(c == 0), stop=(c == NCHUNK - 1))

                # v tile (+ gate logits)
                nc.vector.tensor_copy(out=v_sb[:, tt, 0:HD], in_=qkv_b[:, 128:256])
                nc.vector.tensor_add(glog_g[:, ti, :], qkv_b[:, 256:260], gateb_sb)

                # rope on q (all 4 heads at once via broadcast cos/sin)
                qa3 = qkv_a.rearrange("p (h d) -> p h d", h=NHEAD)
                x1 = qa3[:, :, 0:HALF]
                x2 = qa3[:, :, HALF:HD]
                cos_t = cos_sb[:, tt, :]
                sin_t = sin_sb[:, tt, :]
                cos_b = bass.AP(tensor=cos_t.tensor, offset=cos_t.offset,
                                ap=[cos_t.ap[0], [0, NHEAD], cos_t.ap[1]])
                sin_b = bass.AP(tensor=sin_t.tensor, offset=sin_t.offset,
                                ap=[sin_t.ap[0], [0, NHEAD], sin_t.ap[1]])
                qrot = qst_g[:, ti, :, :]
                u1 = a_sb.tile([128, NHEAD, HALF], F32, tag="u1")
                u2 = a_sb.tile([128, NHEAD, HALF], F32, tag="u2")
                nc.vector.tensor_mul(u1, x1, cos_b)
                nc.vector.tensor_mul(u2, x2, sin_b)
                nc.vector.tensor_add(qrot[:, :, 0:HALF], u1, u2)
                u3 = a_sb.tile([128, NHEAD, HALF], F32, tag="u3")
                u4 = a_sb.tile([128, NHEAD, HALF], F32, tag="u4")
                nc.vector.tensor_mul(u3, x2, cos_b)
                nc.vector.tensor_mul(u4, x1, sin_b)
                nc.vector.tensor_sub(qrot[:, :, HALF:HD], u3, u4)
                # rms scale + gain, cast to bf16
                # rope on k
                k1 = qkv_b[:, 0:HALF]
                k2 = qkv_b[:, HALF:HD]
                krot = kst_g[:, ti, :]
                w1 = a_sb.tile([128, HALF], F32, tag="w1")
                w2 = a_sb.tile([128, HALF], F32, tag="w2")
                nc.vector.tensor_mul(w1, k1, cos_t)
                nc.vector.tensor_mul(w2, k2, sin_t)
                nc.vector.tensor_add(krot[:, 0:HALF], w1, w2)
                nc.vector.tensor_mul(w1, k2, cos_t)
                nc.vector.tensor_mul(w2, k1, sin_t)
                nc.vector.tensor_sub(krot[:, HALF:HD], w1, w2)

                # mean-square per head from the (norm-preserving) rotated values
                sqscr = a_sb.tile([128, NHEAD, HD], F32, tag="sqscr")
                sqscr_k = a_sb.tile([128, HD], F32, tag="sqscr_k")
                nc.vector.tensor_mul(sqscr, qrot, qrot)
                nc.vector.tensor_reduce(msq_g[:, ti, 0:NHEAD], sqscr,
                                        axis=mybir.AxisListType.X,
                                        op=mybir.AluOpType.add)
                nc.vector.tensor_mul(sqscr_k, krot, krot)
                nc.vector.tensor_reduce(msq_g[:, ti, NHEAD:NHEAD + 1], sqscr_k,
                                        axis=mybir.AxisListType.X,
                                        op=mybir.AluOpType.add)

            # batched scalar math for the 4-tile group (one table load each)
            gslice = gate_sb[:, tg * 4:(tg + 1) * 4, :]
            nc.scalar.activation(
                out=gslice.rearrange("p a b -> p (a b)"),
                in_=glog_g.rearrange("p a b -> p (a b)"), func=AF.Sigmoid)
            rtmp_g = a_sb.tile([128, 4, NHEAD + 1], F32, tag="rtmp_g")
            nc.scalar.activation(out=rtmp_g, in_=msq_g, func=AF.Sqrt,
                                 scale=1.0 / HD, bias=eps_sb)
            rinv_g = a_sb.tile([128, 4, NHEAD + 1], F32, tag="rinv_g")
            nc.vector.reciprocal(rinv_g, rtmp_g)
            rq_g = a_sb.tile([128, 4, NHEAD], F32, tag="rq_g")
            for ti in range(4):
                nc.vector.tensor_mul(rq_g[:, ti, :], rinv_g[:, ti, 0:NHEAD],
                                     qgain_sb)

            for ti in range(4):
                tt = tg * 4 + ti
                ts = slice(tt * 128, (tt + 1) * 128)
                k_stage = a_sb.tile([128, HD], BF16, tag="k_stage")
                nc.vector.tensor_scalar_mul(k_stage, kst_g[:, ti, :],
                                            rinv_g[:, ti, NHEAD:NHEAD + 1])
                q_stage = a_sb.tile([128, NHEAD, HD], BF16, tag="q_stage")
                for h in range(NHEAD):
                    nc.vector.tensor_scalar_mul(q_stage[:, h, :],
                                                qst_g[:, ti, h, :],
                                                rq_g[:, ti, h:h + 1])
                # one combined 4-head transpose (strided 3D out)
                nc.sync.dma_start_transpose(out=qT_sb[:, :, ts], in_=q_stage)
                nc.sync.dma_start_transpose(out=kT_sb[:, ts], in_=k_stage)

        # =========== Phase B + C: attention, projection =======================
        b_sb = ctx.enter_context(tc.tile_pool(name="phB", bufs=3))
        c_sb = ctx.enter_context(tc.tile_pool(name="phC", bufs=3))
        with tc.tile_pool(name="phBC_ps", bufs=2, space="PSUM") as b_ps:
            for qc in range(4):
                qs_slice = slice(qc * 512, (qc + 1) * 512)
                nki = 4 * qc + 4
                for h in range(NHEAD):
                    y01 = b_ps.tile([128, 2, HD + 1], F32, tag="y01")
                    y23 = b_ps.tile([128, 2, HD + 1], F32, tag="y23")
                    for ki in range(nki):
                        m = ki - 4 * qc
                        nq = 512 - 128 * max(m, 0)
                        q_lo = qc * 512 + 128 * max(m, 0)
                        s_ps = b_ps.tile([128, 512], F32, tag="s")
                        nc.tensor.matmul(s_ps[:, 0:nq],
                                         lhsT=kT_sb[:, ki * 128:(ki + 1) * 128],
                                         rhs=qT_sb[:, h, q_lo:(qc + 1) * 512],
                                         start=True, stop=True)
                        p_sb = b_sb.tile([128, 512], BF16, tag="p")
                        nc.scalar.activation(out=p_sb[:, 0:nq], in_=s_ps[:, 0:nq],
                                             func=AF.Exp, scale=SM_SCALE)
                        if m >= 0:
                            nc.vector.tensor_mul(p_sb[:, 0:128], p_sb[:, 0:128],
                                                 masks_sb[:, 0, 0:128])
                        for qs in range(max(m, 0), 4):
                            ytile = y01 if qs < 2 else y23
                            pcol = (qs - max(m, 0)) * 128
                            nc.tensor.matmul(
                                ytile[:, qs % 2, :],
                                lhsT=p_sb[:, pcol:pcol + 128],
                                rhs=v_sb[:, ki, :],
                                start=(ki == 0 and qs % 2 == 0),
                                stop=(ki == 4 * qc + qs and qs % 2 == 1))
                    # normalize + gate + transpose (on the Scalar DMA queue,
                    # off the busy Sync queue)
                    y_stage = b_sb.tile([128, 4, HD], BF16, tag="y_stage")
                    for qs in range(4):
                        ytile = y01 if qs < 2 else y23
                        tt = qc * 4 + qs
                        rd = b_sb.tile([128, 1], F32, tag="rd")
                        nc.vector.reciprocal(rd, ytile[:, qs % 2, HD:HD + 1])
                        sc = b_sb.tile([128, 1], F32, tag="sc")
                        nc.vector.tensor_mul(sc, rd, gate_sb[:, tt, h:h + 1])
                        nc.vector.tensor_scalar_mul(y_stage[:, qs, :],
                                                    ytile[:, qs % 2, 0:HD], sc)
                    yreg = yT_sb[:, h, qc * 512:(qc + 1) * 512]
                    y3d = bass.AP(tensor=yreg.tensor, offset=yreg.offset,
                                  ap=[yreg.ap[0], [128, 4], [1, 128]])
                    nc.scalar.dma_start_transpose(out=y3d, in_=y_stage)

                # Phase C for the token tiles finished by this qc
                for qs in range(4):
                    tt = qc * 4 + qs
                    ts = slice(tt * 128, (tt + 1) * 128)
                    for nch in range(4):
                        o_ps = b_ps.tile([128, 512], F32, tag="o")
                        for h in range(NHEAD):
                            nc.tensor.matmul(o_ps, lhsT=yT_sb[:, h, ts],
                                             rhs=wproj_sb[:, h,
                                                          nch * 512:(nch + 1) * 512],
                                             start=(h == 0), stop=(h == NHEAD - 1))
                        o_st = c_sb.tile([128, 512], F32, tag="o_st")
                        if nch % 2 == 0:
                            nc.scalar.copy(out=o_st, in_=o_ps)
                        else:
                            nc.vector.tensor_copy(out=o_st, in_=o_ps)
                        nc.sync.dma_start(out=out_d[ts, nch * 512:(nch + 1) * 512],
                                          in_=o_st)

    nc.compile()
    return nc


def _get_program():
    if "nc" not in _CACHE:
        _CACHE["nc"] = _build_program()
    return _CACHE["nc"]


def _host_prep(x, Wq, Wk, Wv, Wproj, q_gain, gate_w, gate_b):
    """Build the 8 per-core input maps."""
    f = np.float32
    x = np.asarray(x, f)
    WqT = np.asarray(Wq, f).T.astype(NPBF)       # [D, 2048]
    WkT = np.asarray(Wk, f).T.astype(NPBF)       # [D, 512]
    WvT = np.asarray(Wv, f).T.astype(NPBF)
    WpT = np.ascontiguousarray(np.asarray(Wproj, f).T.astype(NPBF))  # [D, D]
    gwT = np.asarray(gate_w, f).T.astype(NPBF)   # [D, 16]
    q_gain = np.asarray(q_gain, f)
    gate_b = np.asarray(gate_b, f)

    inv_freq = 1.0 / (ROPE_BASE ** (np.arange(0, HD, 2, dtype=f) / HD))
    tpos = np.arange(T, dtype=f)
    freqs = np.outer(tpos, inv_freq)
    cos = np.cos(freqs).astype(f)
    sin = np.sin(freqs).astype(f)

    kloc = np.arange(128)[:, None]
    qloc = np.arange(512)[None, :]
    masks = np.stack([(qloc >= kloc + 128 * m) for m in range(4)], axis=1)
    masks = masks.astype(NPBF)                   # [128, 4, 512]

    xT = [np.ascontiguousarray(x[b].T).astype(NPBF) for b in range(B)]

    in_maps = []
    for core in range(8):
        b, g = divmod(core, 4)
        wqkvg = np.concatenate([
            WqT[:, 512 * g:512 * (g + 1)],
            WkT[:, 128 * g:128 * (g + 1)],
            WvT[:, 128 * g:128 * (g + 1)],
            gwT[:, NHEAD * g:NHEAD * (g + 1)],
        ], axis=1)                               # [D, 772]
        in_maps.append({
            "xT": xT[b],
            "wqkvg": np.ascontiguousarray(wqkvg),
            "wproj": np.ascontiguousarray(WpT[512 * g:512 * (g + 1), :]),
            "cosd": cos,
            "sind": sin,
            "qgain": np.ascontiguousarray(q_gain[NHEAD * g:NHEAD * (g + 1)][None, :]),
            "gateb": np.ascontiguousarray(gate_b[NHEAD * g:NHEAD * (g + 1)][None, :]),
            "masks": masks,
        })
    return in_maps


def kernel(**inputs):
    nc = _get_program()
    in_maps = _host_prep(**inputs)
    res = run_bass_kernel_spmd(nc, in_maps, list(range(8)))
    parts = [r["out"] for r in res.results]
    out = np.empty((B, T, D), np.float32)
    for b in range(B):
        out[b] = parts[4 * b] + parts[4 * b + 1] + parts[4 * b + 2] + parts[4 * b + 3]
    return out



# revision 15
# speedup vs baseline: 1.1743x; 1.1743x over previous
"""Trainium2 Bass kernel for CausalSelfAttention (GQA + qk-rmsnorm + rope + head gating).

Sharding: 8 cores = 2 (batch) x 4 (kv-head groups). Each core computes the
full attention for one batch element and one kv-head group (4 q heads), plus
its slice of the output projection; partial projection outputs (bf16) are
summed on the host.

Per-core pipeline (matmuls bf16, fp32 PSUM):
  A) fused QKV+gate projection; rope + per-head mean-square stats computed in
     bf16 on SBUF copies (DVE); per-group batched sqrt/reciprocal; rms scale
     applied to q (with gain) and k; q/k DMA-transposed to head-dim-major
     (one batched transpose per 4-tile group).
  B) flash-style causal attention per (q-chunk, head) in S^T layout:
     S^T = K @ Q^T accumulated into 2-bank PSUM pairs, exp batched over ki
     pairs (one ACT call per 1024 cols when off-diagonal), diagonal-block
     masking on DVE, Y = P @ [V | 1] (ones column = softmax denominator),
     per-token normalize * sigmoid gate, one batched y transpose per q-chunk.
  C) output projection at the end: out = y @ Wproj_slice^T, written as bf16
     partials via the gpsimd (SWDGE) DMA queue.

Phase A tile groups and phase B q-chunks are interleaved in emission order so
the Tensor engine always has matmul work while DVE/ACT run phase-A elementwise
and phase-B exp, keeping the PE HAM clock-gate warm.
"""

import numpy as np
import ml_dtypes
from contextlib import ExitStack

import concourse.bass as bass
import concourse.bacc as bacc
import concourse.mybir as mybir
import concourse.tile as tile
from concourse.bass_utils import run_bass_kernel_spmd

BF16 = mybir.dt.bfloat16
F32 = mybir.dt.float32
NPBF = ml_dtypes.bfloat16

B, T, D = 2, 2048, 2048
H, HKV, HD = 16, 4, 128
HALF = HD // 2
NHEAD = H // HKV          # q heads per core (group)
NT = T // 128             # 16 token tiles
NCHUNK = D // 128         # 16 contraction chunks
NQKV = NHEAD * HD + HD + HD + NHEAD   # 512 q + 128 k + 128 v + 4 gate = 772
ROPE_BASE = 10000.0
EPS = float(np.finfo(np.float32).eps)
SM_SCALE = 1.0 / float(np.sqrt(HD))

_CACHE = {}


def _ap(t, off, dims):
    """Free-dim view of tile t at element offset off with free dims [[stride, n], ...]."""
    return bass.AP(tensor=t.tensor, offset=t.offset + off, ap=[t.ap[0]] + dims)


def _build_program():
    nc = bacc.Bacc("TRN2", target_bir_lowering=False, debug=False,
                   enable_asserts=False, num_devices=8)

    xT_d = nc.dram_tensor("xT", [D, T], BF16, kind="ExternalInput").ap()
    wqkvg_d = nc.dram_tensor("wqkvg", [D, NQKV], BF16, kind="ExternalInput").ap()
    wproj_d = nc.dram_tensor("wproj", [NHEAD * HD, D], BF16, kind="ExternalInput").ap()
    cos5_d = nc.dram_tensor("cos5d", [T, 5 * HALF], BF16, kind="ExternalInput").ap()
    sin5_d = nc.dram_tensor("sin5d", [T, 5 * HALF], BF16, kind="ExternalInput").ap()
    qgain_d = nc.dram_tensor("qgain", [1, NHEAD], F32, kind="ExternalInput").ap()
    gateb_d = nc.dram_tensor("gateb", [1, NHEAD], F32, kind="ExternalInput").ap()
    mask_d = nc.dram_tensor("mask", [128, 128], BF16, kind="ExternalInput").ap()
    out_d = nc.dram_tensor("out", [T, D], BF16, kind="ExternalOutput").ap()

    AF = mybir.ActivationFunctionType
    ALU = mybir.AluOpType

    with tile.TileContext(nc) as tc, ExitStack() as ctx:
        consts = ctx.enter_context(tc.tile_pool(name="consts", bufs=1))

        # ---- small constants first (scalar queue) ----
        cos5_sb = consts.tile([128, NT, 5 * HALF], BF16)
        nc.sync.dma_start(out=cos5_sb,
                            in_=cos5_d.rearrange("(tt p) i -> p tt i", p=128))
        sin5_sb = consts.tile([128, NT, 5 * HALF], BF16)
        nc.sync.dma_start(out=sin5_sb,
                            in_=sin5_d.rearrange("(tt p) i -> p tt i", p=128))
        qgain_sb = consts.tile([128, NHEAD], F32)
        nc.sync.dma_start(out=qgain_sb, in_=bass.AP(
            tensor=qgain_d.tensor, offset=qgain_d.offset,
            ap=[[0, 128], [1, NHEAD]]))
        gateb_sb = consts.tile([128, NHEAD], F32)
        nc.sync.dma_start(out=gateb_sb, in_=bass.AP(
            tensor=gateb_d.tensor, offset=gateb_d.offset,
            ap=[[0, 128], [1, NHEAD]]))
        mask_sb = consts.tile([128, 128], BF16)
        nc.sync.dma_start(out=mask_sb, in_=mask_d)

        # ---- big inputs, interleaved per chunk so tile-0 compute can start
        # as soon as chunk 0 lands (sync queue) ----
        xT_sb = consts.tile([128, NCHUNK, T], BF16)
        wqkvg_sb = consts.tile([128, NCHUNK, NQKV], BF16)
        for c in range(NCHUNK):
            nc.sync.dma_start(out=wqkvg_sb[:, c, :],
                              in_=wqkvg_d[c * 128:(c + 1) * 128, :])
            nc.sync.dma_start(out=xT_sb[:, c, :], in_=xT_d[c * 128:(c + 1) * 128, :])
        wproj_sb = consts.tile([128, NHEAD, D], BF16)
        for h in range(NHEAD):
            nc.sync.dma_start(out=wproj_sb[:, h, :],
                                in_=wproj_d[h * 128:(h + 1) * 128, :])

        # ---- resident state ----
        qT_sb = consts.tile([128, NHEAD, T], BF16)   # head-dim-major q-hat
        kT_sb = consts.tile([128, T], BF16)          # head-dim-major k-hat
        v_sb = consts.tile([128, NT, HD + 1], BF16)  # [v | ones] per ki tile
        nc.vector.memset(v_sb[:, :, HD:HD + 1], 1.0)
        yT_sb = consts.tile([128, NHEAD, T], BF16)   # head-dim-major gated y
        gate_sb = consts.tile([128, NT, NHEAD], F32)
        msq_all = consts.tile([128, NT * 5], F32)    # per-tile sumsq (4 q heads + k)
        rinv_all = consts.tile([128, NT * 5], F32)   # 1/sqrt(msq/HD + eps)
        rq_all = consts.tile([128, NT * NHEAD], F32)  # rinv * q_gain
        eps_sb = consts.tile([128, 1], F32)
        nc.vector.memset(eps_sb, EPS)

        a_sb = tc.alloc_tile_pool(name="phA", bufs=2)
        a_ps = tc.alloc_tile_pool(name="phA_ps", bufs=1, space="PSUM")
        b_sb = tc.alloc_tile_pool(name="phB", bufs=2)
        b_ps = tc.alloc_tile_pool(name="phB_ps", bufs=1, space="PSUM")

        def phase_a_group(tg):
            glog_g = a_sb.tile([128, 4, NHEAD], F32, tag="glog_g")
            rot_keep = []
            for ti in range(4):
                tt = tg * 4 + ti
                ts = slice(tt * 128, (tt + 1) * 128)
                qkv_a = a_ps.tile([128, 512], F32, tag="qkv_a")
                qkv_b = a_ps.tile([128, NQKV - 512], F32, tag="qkv_b")
                for c in range(NCHUNK):
                    lhs = xT_sb[:, c, ts]
                    nc.tensor.matmul(qkv_a, lhsT=lhs, rhs=wqkvg_sb[:, c, 0:512],
                                     start=(c == 0), stop=(c == NCHUNK - 1))
                    nc.tensor.matmul(qkv_b, lhsT=lhs, rhs=wqkvg_sb[:, c, 512:NQKV],
                                     start=(c == 0), stop=(c == NCHUNK - 1))

                # evacuate PSUM -> bf16 SBUF
                qk16 = a_sb.tile([128, 640], BF16, tag="qk16")
                nc.vector.tensor_copy(out=qk16[:, 0:512], in_=qkv_a)
                nc.vector.tensor_copy(out=qk16[:, 512:640], in_=qkv_b[:, 0:128])
                nc.vector.tensor_copy(out=v_sb[:, tt, 0:HD], in_=qkv_b[:, 128:256])
                nc.vector.tensor_add(glog_g[:, ti, :], qkv_b[:, 256:260], gateb_sb)

                # per-head sum-of-squares (pre-rope == post-rope, rotation
                # preserves pairwise norms); 5 groups: 4 q heads + k
                sq_scr = a_sb.tile([128, 5, 128], F32, tag="sq_scr")
                qk5 = qk16.rearrange("p (g i) -> p g i", g=5)
                nc.vector.tensor_mul(sq_scr, qk5, qk5)
                nc.vector.tensor_reduce(
                    msq_all[:, tt * 5:(tt + 1) * 5],
                    sq_scr, axis=mybir.AxisListType.X, op=ALU.add)

                # rope on all 5 groups at once (bf16, dense)
                x1 = _ap(qk16, 0, [[128, 5], [1, HALF]])
                x2 = _ap(qk16, HALF, [[128, 5], [1, HALF]])
                cos_t = cos5_sb[:, tt, :].rearrange("p (g i) -> p g i", g=5)
                sin_t = sin5_sb[:, tt, :].rearrange("p (g i) -> p g i", g=5)
                rot16 = a_sb.tile([128, 640], BF16, tag="rot16", bufs=4)
                r1 = _ap(rot16, 0, [[128, 5], [1, HALF]])
                r2 = _ap(rot16, HALF, [[128, 5], [1, HALF]])
                u1 = a_sb.tile([128, 5, HALF], BF16, tag="u1")
                u2 = a_sb.tile([128, 5, HALF], BF16, tag="u2")
                nc.vector.tensor_mul(u1, x1, cos_t)
                nc.vector.tensor_mul(u2, x2, sin_t)
                nc.vector.tensor_add(r1, u1, u2)
                nc.vector.tensor_mul(u1, x2, cos_t)
                nc.vector.tensor_mul(u2, x1, sin_t)
                nc.vector.tensor_sub(r2, u1, u2)
                rot_keep.append(rot16)

            # batched per-group scalar math
            gslice = gate_sb[:, tg * 4:(tg + 1) * 4, :]
            nc.scalar.activation(
                out=gslice.rearrange("p a b -> p (a b)"),
                in_=glog_g.rearrange("p a b -> p (a b)"), func=AF.Sigmoid)
            rtmp_g = a_sb.tile([128, 20], F32, tag="rtmp_g")
            nc.scalar.activation(out=rtmp_g, in_=msq_all[:, tg * 20:(tg + 1) * 20],
                                 func=AF.Sqrt, scale=1.0 / HD, bias=eps_sb)
            nc.vector.reciprocal(rinv_all[:, tg * 20:(tg + 1) * 20], rtmp_g)
            # rq = rinv(q cols) * gain  (tiles-in-group x 4 heads)
            rinv_q = _ap(rinv_all, tg * 20, [[5, 4], [1, NHEAD]])
            gain_b = bass.AP(tensor=qgain_sb.tensor, offset=qgain_sb.offset,
                             ap=[qgain_sb.ap[0], [0, 4], [1, NHEAD]])
            rq_view = _ap(rq_all, tg * 16, [[4, 4], [1, NHEAD]])
            nc.vector.tensor_mul(rq_view, rinv_q, gain_b)

            # apply rms scales; stage and transpose (q per tile, k per group)
            k_stage = a_sb.tile([128, 4, 128], BF16, tag="k_stage")
            for ti in range(4):
                tt = tg * 4 + ti
                rot16 = rot_keep[ti]
                nc.vector.tensor_scalar_mul(
                    k_stage[:, ti, :], rot16[:, 512:640],
                    rinv_all[:, tt * 5 + 4:tt * 5 + 5])
                q_stage = a_sb.tile([128, NHEAD, 128], BF16, tag="q_stage")
                for h in range(NHEAD):
                    nc.vector.tensor_scalar_mul(
                        q_stage[:, h, :], rot16[:, h * 128:(h + 1) * 128],
                        rq_all[:, tt * 4 + h:tt * 4 + h + 1])
                nc.sync.dma_start_transpose(
                    out=_ap(qT_sb, tt * 128, [[T, NHEAD], [1, 128]]),
                    in_=q_stage)
            nc.sync.dma_start_transpose(
                out=_ap(kT_sb, tg * 512, [[128, 4], [1, 128]]), in_=k_stage)

        def phase_b_qc(qc):
            nki = 4 * qc + 4
            y_stage = b_sb.tile([128, NHEAD, 4, HD], BF16, tag="y_stage")
            for h in range(NHEAD):
                y01 = b_ps.tile([128, 2, HD + 1], F32, tag="y01")
                y23 = b_ps.tile([128, 2, HD + 1], F32, tag="y23")
                for pair in range(nki // 2):
                    s2 = b_ps.tile([128, 2, 512], F32, tag="s2", bufs=2)
                    p2 = b_sb.tile([128, 2, 512], BF16, tag="p2", bufs=4)
                    for half in (0, 1):
                        ki = 2 * pair + half
                        m = ki - 4 * qc
                        nq = 512 - 128 * max(m, 0)
                        q_lo = qc * 512 + 128 * max(m, 0)
                        nc.tensor.matmul(s2[:, half, 0:nq],
                                         lhsT=kT_sb[:, ki * 128:(ki + 1) * 128],
                                         rhs=qT_sb[:, h, q_lo:(qc + 1) * 512],
                                         start=True, stop=True)
                    for half in (0, 1):
                        ki = 2 * pair + half
                        nq = 512 - 128 * max(ki - 4 * qc, 0)
                        nc.scalar.activation(out=p2[:, half, 0:nq],
                                             in_=s2[:, half, 0:nq],
                                             func=AF.Exp, scale=SM_SCALE)
                    for half in (0, 1):
                        ki = 2 * pair + half
                        m = ki - 4 * qc
                        if m >= 0:
                            nc.vector.tensor_mul(p2[:, half, 0:128],
                                                 p2[:, half, 0:128], mask_sb)
                        for qs in range(max(m, 0), 4):
                            ytile = y01 if qs < 2 else y23
                            pcol = (qs - max(m, 0)) * 128
                            nc.tensor.matmul(
                                ytile[:, qs % 2, :],
                                lhsT=p2[:, half, pcol:pcol + 128],
                                rhs=v_sb[:, ki, :],
                                start=(ki == 0 and qs % 2 == 0),
                                stop=(ki == 4 * qc + qs and qs % 2 == 1))
                # normalize + gate into y_stage
                for pair01, ytile in ((0, y01), (1, y23)):
                    rd = b_sb.tile([128, 2], F32, tag="rd")
                    nc.vector.reciprocal(
                        rd, _ap(ytile, HD, [[HD + 1, 2]]))
                    sc = b_sb.tile([128, 2], F32, tag="sc")
                    g_ap = _ap(gate_sb, (qc * 4 + pair01 * 2) * NHEAD + h,
                               [[NHEAD, 2]])
                    nc.vector.tensor_mul(sc, rd, g_ap)
                    for j in (0, 1):
                        qs = pair01 * 2 + j
                        nc.vector.tensor_scalar_mul(
                            y_stage[:, h, qs, :], ytile[:, j, 0:HD],
                            sc[:, j:j + 1])
            for h in range(NHEAD):
                nc.sync.dma_start_transpose(
                    out=_ap(yT_sb, h * T + qc * 512, [[128, 4], [1, 128]]),
                    in_=y_stage[:, h])

        # ---- interleaved emission: A groups feed B q-chunks ----
        phase_a_group(0)
        phase_a_group(1)
        phase_b_qc(0)
        phase_a_group(2)
        phase_b_qc(1)
        phase_a_group(3)
        phase_b_qc(2)
        phase_b_qc(3)

        # release A/B pools before C so PSUM/SBUF fit
        b_ps.release()
        a_ps.release()
        b_sb.release()
        a_sb.release()

        # ================= Phase C: output projection =================
        c_sb = ctx.enter_context(tc.tile_pool(name="phC", bufs=2))
        with tc.tile_pool(name="phC_ps", bufs=3, space="PSUM") as c_ps:
            for tt in range(NT):
                ts = slice(tt * 128, (tt + 1) * 128)
                o16 = c_sb.tile([128, D], BF16, tag="o16")
                for nchp in range(2):
                    o2 = c_ps.tile([128, 2, 512], F32, tag="o2")
                    for half in (0, 1):
                        cols = slice((2 * nchp + half) * 512,
                                     (2 * nchp + half + 1) * 512)
                        for h in range(NHEAD):
                            nc.tensor.matmul(o2[:, half, :],
                                             lhsT=yT_sb[:, h, ts],
                                             rhs=wproj_sb[:, h, cols],
                                             start=(h == 0), stop=(h == NHEAD - 1))
                    for half in (0, 1):
                        nc.vector.tensor_copy(
                            out=o16[:, (2 * nchp + half) * 512:
                                    (2 * nchp + half + 1) * 512],
                            in_=o2[:, half, :])
                nc.sync.dma_start(out=out_d[ts, :], in_=o16)

    nc.compile()
    return nc


def _get_program():
    if "nc" not in _CACHE:
        _CACHE["nc"] = _build_program()
    return _CACHE["nc"]


def _host_prep(x, Wq, Wk, Wv, Wproj, q_gain, gate_w, gate_b):
    """Build the 8 per-core input maps."""
    f = np.float32
    x = np.asarray(x, f)
    WqT = np.asarray(Wq, f).T.astype(NPBF)       # [D, 2048]
    WkT = np.asarray(Wk, f).T.astype(NPBF)       # [D, 512]
    WvT = np.asarray(Wv, f).T.astype(NPBF)
    WpT = np.ascontiguousarray(np.asarray(Wproj, f).T.astype(NPBF))  # [D, D]
    gwT = np.asarray(gate_w, f).T.astype(NPBF)   # [D, 16]
    q_gain = np.asarray(q_gain, f)
    gate_b = np.asarray(gate_b, f)

    inv_freq = 1.0 / (ROPE_BASE ** (np.arange(0, HD, 2, dtype=f) / HD))
    tpos = np.arange(T, dtype=f)
    freqs = np.outer(tpos, inv_freq)
    cos5 = np.ascontiguousarray(np.tile(np.cos(freqs), (1, 5)).astype(NPBF))
    sin5 = np.ascontiguousarray(np.tile(np.sin(freqs), (1, 5)).astype(NPBF))

    kloc = np.arange(128)[:, None]
    qloc = np.arange(128)[None, :]
    mask = (qloc >= kloc).astype(NPBF)           # [128, 128]

    xT = [np.ascontiguousarray(x[b].T).astype(NPBF) for b in range(B)]

    in_maps = []
    for core in range(8):
        b, g = divmod(core, 4)
        wqkvg = np.concatenate([
            WqT[:, 512 * g:512 * (g + 1)],
            WkT[:, 128 * g:128 * (g + 1)],
            WvT[:, 128 * g:128 * (g + 1)],
            gwT[:, NHEAD * g:NHEAD * (g + 1)],
        ], axis=1)                               # [D, 772]
        in_maps.append({
            "xT": xT[b],
            "wqkvg": np.ascontiguousarray(wqkvg),
            "wproj": np.ascontiguousarray(WpT[512 * g:512 * (g + 1), :]),
            "cos5d": cos5,
            "sin5d": sin5,
            "qgain": np.ascontiguousarray(q_gain[NHEAD * g:NHEAD * (g + 1)][None, :]),
            "gateb": np.ascontiguousarray(gate_b[NHEAD * g:NHEAD * (g + 1)][None, :]),
            "mask": mask,
        })
    return in_maps


def kernel(**inputs):
    nc = _get_program()
    in_maps = _host_prep(**inputs)
    res = run_bass_kernel_spmd(nc, in_maps, list(range(8)))
    parts = [r["out"].astype(np.float32) for r in res.results]
    out = np.empty((B, T, D), np.float32)
    for b in range(B):
        out[b] = parts[4 * b] + parts[4 * b + 1] + parts[4 * b + 2] + parts[4 * b + 3]
    return out
